# revision 6
# baseline (speedup 1.0000x reference)
"""BayesianDeeperGCN on 8 TRN2 NeuronCores (Bass/Tile).

Sharding: nodes across 8 cores (6250 each, padded to 6272 = 49*128);
edges partitioned by destination owner; per layer an AllGather of the
per-node softmax-message table [P|Q] = [exp(t+eps) | t*exp(t+eps)],
then a dma_gather + indicator-matmul segmented reduction per dst tile.

Key identity: GENConv softmax aggregation
  agg[d] = sum_e exp(m_e)*m_e / sum_e exp(m_e),  m_e = t[src_e] + eps
(t = relu(LN(h)) >= 0 so no max-subtraction needed) which makes the
whole edge computation two gather-scatter sums of *node* tables P,Q.
"""
import os
import sys
import types
import contextlib
import ctypes
import numpy as np
import ml_dtypes

import concourse.bass as bass
import concourse.bacc as bacc
import concourse.tile as tile
import concourse.mybir as mybir
from concourse.bass_utils import run_bass_kernel_spmd

BF16 = ml_dtypes.bfloat16

N = 50000
NC = 8
PN = N // NC            # 6250 nodes per core
NT = 49                 # node tiles of 128 per core
PADN = NT * 128         # 6272
TBLR = NC * PADN        # 50176 padded table rows
LO = 32768              # int16 index split
H = 128
H2 = 256
L = 3
WIN = 4                 # dst tiles per PSUM window
CAP = 16                 # max chunks per dma_gather batch
EPS_GEN = 1e-7
EPS_NORM = 1e-5
NCHUNKS_512 = [(i * 512, 512) for i in range(12)] + [(6144, 128)]


def _install_profile_shim():
    if "antenv.axon_hooks" in sys.modules:
        return
    so = "/opt/axon/libaxon_pjrt.so"
    hook = None
    if os.path.exists(so):
        lib = ctypes.CDLL(so)
        if hasattr(lib, "axon_start_nrt_profile"):
            lib.axon_start_nrt_profile.argtypes = [ctypes.POINTER(ctypes.c_int64), ctypes.c_size_t]
            lib.axon_start_nrt_profile.restype = ctypes.c_int64
            lib.axon_stop_nrt_profile.argtypes = [ctypes.c_char_p]
            lib.axon_stop_nrt_profile.restype = ctypes.c_int64

            @contextlib.contextmanager
            def _hook(output_dir, device_ids):
                import jax
                jax.devices()
                if device_ids:
                    ids = (ctypes.c_int64 * len(device_ids))(*device_ids)
                    rc = lib.axon_start_nrt_profile(ids, len(device_ids))
                else:
                    rc = lib.axon_start_nrt_profile(None, 0)
                if rc != 0:
                    raise RuntimeError(f"axon_start_nrt_profile rc={rc}")
                try:
                    yield
                finally:
                    n = lib.axon_stop_nrt_profile(str(output_dir).encode())
                    print(f"profile: {n} file(s) -> {output_dir}", file=sys.stderr)

            hook = _hook
    mod = types.ModuleType("antenv.axon_hooks")
    mod.get_axon_ntff_profile_hook = lambda: hook
    mod.set_axon_ntff_profile_hook = lambda h: None
    sys.modules["antenv.axon_hooks"] = mod
    try:
        import antenv
        antenv.axon_hooks = mod
    except ImportError:
        pass


# ---------------------------------------------------------------- host prep

class Prep:
    pass


def preprocess(edge_index):
    src = edge_index[0].astype(np.int64)
    dst = edge_index[1].astype(np.int64)
    owner = dst // PN
    prow = (src // PN) * PADN + (src % PN)

    counts = np.zeros((NC, NT, 2), np.int64)
    per_core = []
    for o in range(NC):
        m = owner == o
        s = prow[m]
        dl = dst[m] - o * PN
        t = dl // 128
        d128 = dl % 128
        g = (s >= LO).astype(np.int64)
        per_core.append((s, t, d128, g))
        for tt in range(NT):
            tm = t == tt
            counts[o, tt, 0] = int(np.sum(tm & (g == 0)))
            counts[o, tt, 1] = int(np.sum(tm & (g == 1)))

    C = np.ceil(counts.max(axis=0) / 128.0).astype(np.int64)  # [NT,2]

    # global chunk order: window-major, grp, tile, chunk
    chunk_meta = []   # (tile, grp)
    slot_base = {}    # (tile,grp) -> first global chunk idx
    for w0 in range(0, NT, WIN):
        tiles = list(range(w0, min(w0 + WIN, NT)))
        for g in (0, 1):
            for t in tiles:
                slot_base[(t, g)] = len(chunk_meta)
                for _ in range(C[t, g]):
                    chunk_meta.append((t, g))
    NCHT = len(chunk_meta)
    NIDX = NCHT * 128

    # first/last chunk per tile (start/stop flags)
    first_chunk = {}
    last_chunk = {}
    for q, (t, g) in enumerate(chunk_meta):
        if t not in first_chunk:
            first_chunk[t] = q
        last_chunk[t] = q

    # batches: consecutive chunks, same grp, same window, <= CAP
    batches = []  # (grp, q0, nq, window_idx)
    q = 0
    while q < NCHT:
        t0, g0 = chunk_meta[q]
        w = t0 // WIN
        q1 = q
        while (q1 < NCHT and chunk_meta[q1][1] == g0
               and chunk_meta[q1][0] // WIN == w and q1 - q < CAP):
            q1 += 1
        batches.append((g0, q, q1 - q, w))
        q = q1

    idx_all = np.zeros((NC, NIDX), np.int16)
    dstc = np.full((NC, NCHT, 128), 200.0, np.float32)
    for o in range(NC):
        s, t, d128, g = per_core[o]
        for tt in range(NT):
            for gg in (0, 1):
                sel = (t == tt) & (g == gg)
                k = int(sel.sum())
                if k == 0:
                    continue
                base = slot_base[(tt, gg)] * 128
                vals = s[sel] - (LO if gg else 0)
                idx_all[o, base:base + k] = vals.astype(np.int16)
                blk = dstc[o].reshape(-1)
                blk[base:base + k] = d128[sel].astype(np.float32)

    p = Prep()
    p.C = C
    p.chunk_meta = chunk_meta
    p.first_chunk = first_chunk
    p.last_chunk = last_chunk
    p.batches = batches
    p.NCHT = NCHT
    p.NIDX = NIDX
    # wrapped idx layout [128, NIDX/16]
    wrapped = np.zeros((NC, 128, NIDX // 16), np.int16)
    for o in range(NC):
        w16 = idx_all[o].reshape(NIDX // 16, 16).T
        wrapped[o] = np.tile(w16, (8, 1))
    p.idx_wrapped = wrapped
    p.dstcol = np.transpose(dstc, (0, 2, 1)).copy()  # [NC,128,NCHT]
    return p


# ---------------------------------------------------------------- kernel build

def build(prep, has_ln_aff):
    CUT = int(os.environ.get("KERNEL_CUT", "0"))
    f32 = mybir.dt.float32
    bf = mybir.dt.bfloat16
    nc = bacc.Bacc(None, num_swdge_queues=1, dynamic_dma_scratch_size=32768)
    AF = mybir.ActivationFunctionType
    OP = mybir.AluOpType

    def din(name, shape, dt=f32):
        return nc.dram_tensor(name, shape, dt, kind="ExternalInput")

    xT_in = din("xT", [128, PADN], bf)
    idx_in = din("idx", [128, prep.NIDX // 16], mybir.dt.int16)
    dstc_in = din("dstc", [128, prep.NCHT])
    iota_in = din("iota", [128, 128])
    idbf_in = din("idbf", [128, 128], bf)
    idf_in = din("idf", [128, 128])
    encw_in = din("encw", [128, 128], bf)
    encb_in = din("encb", [128, 1])
    w1_in = din("w1", [128, L * 256], bf)
    b1_in = din("b1c", [128, L * 2])
    b1p_in = din("b1p", [128, L * 2])
    bng_in = din("bng", [128, L * 2])
    bnb_in = din("bnb", [128, L * 2])
    w2_in = din("w2", [128, L * 256], bf)
    b2_in = din("b2c", [128, L])
    lng_in = din("lng", [128, L * 128])
    lnb_in = din("lnb", [128, L * 128])
    clfw_in = din("clfw", [128, 64])
    clfb_in = din("clfb", [64, 1])
    out_ext = nc.dram_tensor("out", [64, PADN], f32, kind="ExternalOutput")

    bounce = nc.dram_tensor("bounce", [PADN, 256], bf)
    table = nc.dram_tensor("table", [TBLR, 256], bf, addr_space="Shared")
    bnloc = nc.dram_tensor("bnloc", [128, 4], f32)
    zrows = nc.dram_tensor("zrows", [22, 128], f32, kind="ExternalInput")
    bnred = nc.dram_tensor("bnred", [128, 4], f32, addr_space="Shared")

    RG = [list(range(NC))]

    with tile.TileContext(nc) as tc:
        with (
            tc.tile_pool(name="const", bufs=1) as cpool,
            tc.tile_pool(name="state", bufs=1) as spool,
            tc.tile_pool(name="gathp", bufs=3) as gpool,
            tc.tile_pool(name="indp", bufs=3) as ipool,
            tc.tile_pool(name="work", bufs=3) as wpool,
            tc.tile_pool(name="tiny", bufs=2) as ypool,
            tc.tile_pool(name="agg", bufs=WIN, space="PSUM") as agg_pool,
            tc.tile_pool(name="mm", bufs=2, space="PSUM") as mm_pool,
            tc.tile_pool(name="tr", bufs=2, space="PSUM") as tr_pool,
        ):
            def load(src_ap, shape, dt=f32, pool=cpool, name=None):
                t = pool.tile(shape, dt, name=name, tag=name)
                nc.sync.dma_start(t[:], src_ap)
                return t

            idx = load(idx_in[:], [128, prep.NIDX // 16], mybir.dt.int16, name="idx")
            dstc = load(dstc_in[:], [128, prep.NCHT], name="dstc")
            iota = load(iota_in[:], [128, 128], name="iota")
            idbf = load(idbf_in[:], [128, 128], bf, name="idbf")
            idf = load(idf_in[:], [128, 128], name="idf")
            encw = load(encw_in[:], [128, 128], bf, name="encw")
            encb = load(encb_in[:], [128, 1], name="encb")
            w1 = load(w1_in[:], [128, L * 256], bf, name="w1")
            b1c = load(b1_in[:], [128, L * 2], name="b1c")
            b1p = load(b1p_in[:], [128, L * 2], name="b1p")
            bng = load(bng_in[:], [128, L * 2], name="bng")
            bnb = load(bnb_in[:], [128, L * 2], name="bnb")
            w2 = load(w2_in[:], [128, L * 256], bf, name="w2")
            b2c = load(b2_in[:], [128, L], name="b2c")
            lng = load(lng_in[:], [128, L * 128], name="lng") if has_ln_aff else None
            lnb = load(lnb_in[:], [128, L * 128], name="lnb") if has_ln_aff else None
            clfw = load(clfw_in[:], [128, 64], name="clfw")
            clfb = load(clfb_in[:], [64, 1], name="clfb")

            xT = spool.tile([128, 2 * PADN], bf, tag="h2T", name="xT")
            nc.sync.dma_start(xT[:, 0:PADN], xT_in[:])
            epsn_c = cpool.tile([128, 1], f32)
            nc.vector.memset(epsn_c[:], EPS_NORM)
            epsg_c = cpool.tile([128, 1], f32)
            nc.vector.memset(epsg_c[:], EPS_GEN)
            h = spool.tile([128, PADN], f32)
            tbuf = spool.tile([128, PADN], f32, tag="tbuf")
            pq = spool.tile([128, NT * 256], bf)
            hc = spool.tile([128, PADN], bf)
            hcT = spool.tile([128, PADN], bf)
            h2T = spool.tile([128, 2 * PADN], bf)
            stats6 = spool.tile([128, NT * 6], f32)

            # ---------------- encoder: h0T = encw.T @ xT ; transpose to h
            sc_enc = nc.enter_named_scope("enc", False)
            for (c0, cn) in NCHUNKS_512:
                ps = mm_pool.tile([128, 512], f32, tag="mm")
                nc.tensor.matmul(ps[:, 0:cn], encw[:], xT[:, 0:PADN][:, c0:c0 + cn],
                                 start=True, stop=True)
                nc.vector.tensor_scalar(tbuf[:, c0:c0 + cn], ps[:, 0:cn],
                                        encb[:, 0:1], None, OP.add)
            for t in range(NT):
                pt = tr_pool.tile([128, 128], f32, tag="tr")
                nc.tensor.transpose(pt[:], tbuf[:, t * 128:(t + 1) * 128], idf[:])
                nc.vector.tensor_copy(h[:, t * 128:(t + 1) * 128], pt[:])
            nc.sync.dma_start(h[106:128, 48 * 128:PADN], zrows[:])
            nc.leave_named_scope("enc", sc_enc[0], False)

            # ---------------- layers
            NL = 0 if CUT == 1 else (1 if CUT in (2, 3) else L)
            for li in range(NL):
                sc_a = nc.enter_named_scope(f"L{li}_ln_pq", False)
                # ---- A1: LN stats per tile
                for t in range(NT):
                    nc.vector.bn_stats(stats6[:, t * 6:(t + 1) * 6],
                                       h[:, t * 128:(t + 1) * 128])
                # ---- A2: batched mu/rstd
                def sview(k):
                    return (stats6[:].rearrange("p (t s) -> p t s", s=6)
                            [:, :, k:k + 1].rearrange("p t s -> p (t s)"))
                me, mo, m2e, m2o = sview(1), sview(4), sview(2), sview(5)
                musum = ypool.tile([128, NT], f32, tag="y1")
                nc.vector.tensor_tensor(musum[:], me, mo, OP.add)
                mu_all = ypool.tile([128, NT], f32, tag="y2")
                nc.vector.tensor_scalar(mu_all[:], musum[:], 0.5, None, OP.mult)
                mesq = ypool.tile([128, NT], f32, tag="y3")
                nc.vector.tensor_tensor(mesq[:], me, me, OP.mult)
                mosq = ypool.tile([128, NT], f32, tag="y4")
                nc.vector.tensor_tensor(mosq[:], mo, mo, OP.mult)
                sqs = ypool.tile([128, NT], f32, tag="y5")
                nc.vector.tensor_tensor(sqs[:], mesq[:], mosq[:], OP.add)
                m2s = ypool.tile([128, NT], f32, tag="y3")
                nc.vector.tensor_tensor(m2s[:], m2e, m2o, OP.add)
                # sumsq = m2s + 64*sqs ; E2 = sumsq/128 ; var = E2 - mu^2
                sq64 = ypool.tile([128, NT], f32, tag="y4")
                nc.vector.tensor_scalar(sq64[:], sqs[:], 64.0, None, OP.mult)
                ssq = ypool.tile([128, NT], f32, tag="y5")
                nc.vector.tensor_tensor(ssq[:], m2s[:], sq64[:], OP.add)
                musq = ypool.tile([128, NT], f32, tag="y3")
                nc.vector.tensor_tensor(musq[:], mu_all[:], mu_all[:], OP.mult)
                var = ypool.tile([128, NT], f32, tag="y4")
                nc.vector.tensor_scalar(var[:], ssq[:], 1.0 / 128, None, OP.mult)
                nc.vector.tensor_tensor(var[:], var[:], musq[:], OP.subtract)
                lnv = ypool.tile([128, NT], f32, tag="y5")
                nc.scalar.activation(lnv[:], var[:], AF.Ln, bias=epsn_c[:, 0:1])
                lnv2 = ypool.tile([128, NT], f32, tag="y3")
                nc.vector.tensor_scalar(lnv2[:], lnv[:], -0.5, None, OP.mult)
                rstd = ypool.tile([128, NT], f32, tag="y6")
                nc.scalar.activation(rstd[:], lnv2[:], AF.Exp)

                # ---- A3: wide t = relu(norm), then per-tile P/Q
                muv = mu_all[:].rearrange("p (t o) -> p t o", o=1).broadcast_to((128, NT, 128))
                rsv = rstd[:].rearrange("p (t o) -> p t o", o=1).broadcast_to((128, NT, 128))
                hv = h[:].rearrange("p (t j) -> p t j", j=128)
                tv = tbuf[:].rearrange("p (t j) -> p t j", j=128)
                nc.vector.tensor_tensor(tv, hv, muv, OP.subtract)
                nc.vector.tensor_tensor(tv, tv, rsv, OP.mult)
                if has_ln_aff:
                    lgv = (lng[:, li * 128:(li + 1) * 128]
                           .rearrange("p (o j) -> p o j", o=1).broadcast_to((128, NT, 128)))
                    lbv = (lnb[:, li * 128:(li + 1) * 128]
                           .rearrange("p (o j) -> p o j", o=1).broadcast_to((128, NT, 128)))
                    nc.vector.tensor_tensor(tv, tv, lgv, OP.mult)
                    nc.vector.tensor_tensor(tv, tv, lbv, OP.add)
                nc.vector.tensor_scalar(tbuf[:], tbuf[:], 0.0, None, OP.max)
                for t in range(NT):
                    ts_ = tbuf[:, t * 128:(t + 1) * 128]
                    p32 = wpool.tile([128, 128], f32, tag="p32")
                    nc.scalar.activation(p32[:], ts_, AF.Exp, bias=epsg_c[:, 0:1])
                    nc.vector.tensor_copy(pq[:, t * 256:t * 256 + 128], p32[:])
                    nc.vector.tensor_tensor(pq[:, t * 256 + 128:t * 256 + 256],
                                            p32[:], ts_, OP.mult)

                nc.leave_named_scope(f"L{li}_ln_pq", sc_a[0], False)
                # ---- B: DMA pq -> bounce, AllGather
                sc_b = nc.enter_named_scope(f"L{li}_ag", False)
                pqv = pq[:].rearrange("p (t c) -> p t c", c=256)
                bv = bounce[:].rearrange("(t p) c -> p t c", p=128)
                nc.gpsimd.dma_start(bv, pqv)
                nc.gpsimd.collective_compute(
                    "AllGather", OP.bypass, replica_groups=RG,
                    ins=[bounce[:].opt()], outs=[table[:].opt()])
                nc.leave_named_scope(f"L{li}_ag", sc_b[0], False)

                # ---- C: edge phase
                if CUT == 2:
                    break
                sc_c = nc.enter_named_scope(f"L{li}_edge", False)
                lo_view = table[0:LO, :]
                hi_view = table[LO:TBLR, :]
                aggps = {}
                cur_w = -1
                EN = int(os.environ.get("KERNEL_EDGE_N", "0"))
                EMODE = os.environ.get("KERNEL_EDGE_MODE", "all")
                ebatches = prep.batches[:EN] if EN else prep.batches
                for bi, (g0, q0, nq, w) in enumerate(ebatches):
                    if w != cur_w:
                        # drain previous window
                        if cur_w >= 0 and EMODE == "all" and not EN:
                            for t in range(cur_w * WIN, min((cur_w + 1) * WIN, NT)):
                                _drain(nc, tc, wpool, aggps[t], tbuf, hc, t)
                        aggps = {}
                        cur_w = w
                        for t in range(w * WIN, min((w + 1) * WIN, NT)):
                            aggps[t] = agg_pool.tile([128, 256], f32, tag="agg", name=f"agg{t % WIN}")
                    gb = gpool.tile([128, nq, 256], bf, tag="gath")
                    src_view = hi_view if g0 else lo_view
                    if EMODE in ("all", "gather"):
                        nc.gpsimd.dma_gather(gb[:], src_view, idx[:, q0 * 8:(q0 + nq) * 8],
                                             nq * 128, nq * 128, 256,
                                             single_packet=False, queue_num=0)
                    ib = ipool.tile([128, nq * 128], bf, tag="ind")
                    if EMODE in ("all", "ind"):
                        dv = (dstc[:, q0:q0 + nq].rearrange("p (q o) -> p q o", o=1)
                              .broadcast_to((128, nq, 128)))
                        iv = (iota[:].rearrange("p (o j) -> p o j", o=1)
                              .broadcast_to((128, nq, 128)))
                        ibv = ib[:].rearrange("p (q j) -> p q j", j=128)
                        nc.vector.tensor_tensor(ibv, dv, iv, OP.is_equal)
                    if EMODE == "all":
                        for ci in range(nq):
                            q = q0 + ci
                            t = prep.chunk_meta[q][0]
                            nc.tensor.matmul(aggps[t][:], ib[:, ci * 128:(ci + 1) * 128],
                                             gb[:, ci, :],
                                             start=(q == prep.first_chunk[t]),
                                             stop=(q == prep.last_chunk[t]))
                if EMODE == "all" and not EN:
                    for t in range(cur_w * WIN, min((cur_w + 1) * WIN, NT)):
                        _drain(nc, tc, wpool, aggps[t], tbuf, hc, t)
                nc.leave_named_scope(f"L{li}_edge", sc_c[0], False)

                # ---- D: dense tail
                if CUT == 3:
                    break
                sc_d = nc.enter_named_scope(f"L{li}_tail", False)
                for t in range(NT):
                    pt = tr_pool.tile([128, 128], bf, tag="tr", name="ptb")
                    nc.tensor.transpose(pt[:], hc[:, t * 128:(t + 1) * 128], idbf[:])
                    nc.vector.tensor_copy(hcT[:, t * 128:(t + 1) * 128], pt[:])
                accC = ypool.tile([128, 26], f32, tag="acc")
                for hf in range(2):
                    lhs = w1[:, li * 256 + hf * 128: li * 256 + (hf + 1) * 128]
                    for i, (c0, cn) in enumerate(NCHUNKS_512):
                        ps = mm_pool.tile([128, 512], f32, tag="mm")
                        nc.tensor.matmul(ps[:, 0:cn], lhs, hcT[:, c0:c0 + cn],
                                         start=True, stop=True)
                        nc.vector.tensor_scalar(
                            h2T[:, hf * PADN + c0: hf * PADN + c0 + cn],
                            ps[:, 0:cn], b1c[:, li * 2 + hf: li * 2 + hf + 1],
                            0.0, OP.add, OP.add,
                            accum_out=accC[:, hf * 13 + i: hf * 13 + i + 1])
                    nc.vector.memset(h2T[:, hf * PADN + 6250: (hf + 1) * PADN], 0.0)
                # BN stats
                st4 = ypool.tile([128, 4], f32, tag="st4")
                for hf in range(2):
                    sumh = ypool.tile([128, 1], f32, tag="y1")
                    nc.vector.tensor_reduce(sumh[:], accC[:, hf * 13:(hf + 1) * 13],
                                            mybir.AxisListType.X, OP.add)
                    nc.vector.tensor_tensor(st4[:, hf:hf + 1], sumh[:],
                                            b1p[:, li * 2 + hf: li * 2 + hf + 1],
                                            OP.subtract)
                    nc.scalar.activation(hcT[:], h2T[:, hf * PADN:(hf + 1) * PADN],
                                         AF.Square,
                                         accum_out=st4[:, 2 + hf:3 + hf])
                nc.sync.dma_start(bnloc[:], st4[:])
                nc.gpsimd.collective_compute(
                    "AllReduce", OP.add, replica_groups=RG,
                    ins=[bnloc[:].opt()], outs=[bnred[:].opt()])
                gst = ypool.tile([128, 4], f32, tag="gst")
                nc.sync.dma_start(gst[:], bnred[:])
                for hf in range(2):
                    mu = ypool.tile([128, 1], f32, tag="y1")
                    nc.vector.tensor_scalar(mu[:], gst[:, hf:hf + 1], 1.0 / N, None, OP.mult)
                    musq = ypool.tile([128, 1], f32, tag="y2")
                    nc.vector.tensor_tensor(musq[:], mu[:], mu[:], OP.mult)
                    var = ypool.tile([128, 1], f32, tag="y3")
                    nc.vector.tensor_scalar(var[:], gst[:, 2 + hf:3 + hf], 1.0 / N,
                                            musq[:], OP.mult, OP.subtract)
                    lnv = ypool.tile([128, 1], f32, tag="y4")
                    nc.scalar.activation(lnv[:], var[:], AF.Ln, bias=epsn_c[:, 0:1])
                    lnv2 = ypool.tile([128, 1], f32, tag="y5")
                    nc.vector.tensor_scalar(lnv2[:], lnv[:], -0.5, None, OP.mult)
                    rs = ypool.tile([128, 1], f32, tag="y4")
                    nc.scalar.activation(rs[:], lnv2[:], AF.Exp)
                    a = ypool.tile([128, 1], f32, tag="y5")
                    nc.vector.tensor_tensor(a[:], bng[:, li * 2 + hf: li * 2 + hf + 1],
                                            rs[:], OP.mult)
                    amu = ypool.tile([128, 1], f32, tag="y4")
                    nc.vector.tensor_tensor(amu[:], a[:], mu[:], OP.mult)
                    bp = ypool.tile([128, 1], f32, tag="y6")
                    nc.vector.tensor_tensor(bp[:], bnb[:, li * 2 + hf: li * 2 + hf + 1],
                                            amu[:], OP.subtract)
                    nc.scalar.activation(h2T[:, hf * PADN:(hf + 1) * PADN],
                                         h2T[:, hf * PADN:(hf + 1) * PADN],
                                         AF.Relu, bias=bp[:, 0:1], scale=a[:, 0:1])
                # w2
                for i, (c0, cn) in enumerate(NCHUNKS_512):
                    ps = mm_pool.tile([128, 512], f32, tag="mm")
                    for cc in range(2):
                        lhs = w2[:, li * 256 + cc * 128: li * 256 + (cc + 1) * 128]
                        nc.tensor.matmul(ps[:, 0:cn], lhs,
                                         h2T[:, cc * PADN + c0: cc * PADN + c0 + cn],
                                         start=(cc == 0), stop=(cc == 1))
                    nc.vector.tensor_scalar(tbuf[:, c0:c0 + cn], ps[:, 0:cn],
                                            b2c[:, li:li + 1], None, OP.add)
                for t in range(NT):
                    pt = tr_pool.tile([128, 128], f32, tag="tr")
                    nc.tensor.transpose(pt[:], tbuf[:, t * 128:(t + 1) * 128], idf[:])
                    nc.vector.tensor_tensor(h[:, t * 128:(t + 1) * 128],
                                            h[:, t * 128:(t + 1) * 128], pt[:], OP.add)
                nc.sync.dma_start(h[106:128, 48 * 128:PADN], zrows[:])
                nc.leave_named_scope(f"L{li}_tail", sc_d[0], False)

            # ---------------- classifier
            for t in range(NT):
                pt = tr_pool.tile([128, 128], f32, tag="tr")
                nc.tensor.transpose(pt[:], h[:, t * 128:(t + 1) * 128], idf[:])
                nc.vector.tensor_copy(tbuf[:, t * 128:(t + 1) * 128], pt[:])
            ob = spool.tile([64, PADN], f32, tag="pq", name="ob")
            for (c0, cn) in NCHUNKS_512:
                ps = mm_pool.tile([64, 512], f32, tag="mm", name="psclf")
                nc.tensor.matmul(ps[:, 0:cn], clfw[:], tbuf[:, c0:c0 + cn],
                                 start=True, stop=True)
                nc.vector.tensor_scalar(ob[:, c0:c0 + cn], ps[:, 0:cn],
                                        clfb[:, 0:1], None, OP.add)
            nc.sync.dma_start(out_ext[:], ob[:])
    nc.compile()
    return nc


def _drain(nc, tc, wpool, ps, tbuf, hc, t):
    OP = mybir.AluOpType
    f32 = mybir.dt.float32
    den = wpool.tile([128, 128], f32, tag="den")
    nc.vector.tensor_scalar(den[:], ps[:, 0:128], 1e-20, None, OP.add)
    r = wpool.tile([128, 128], f32, tag="rcp")
    nc.vector.reciprocal_approx_fast(r[:], den[:])
    qn = wpool.tile([128, 128], f32, tag="qn")
    nc.vector.tensor_tensor(qn[:], ps[:, 128:256], r[:], OP.mult)
    nc.vector.tensor_tensor(hc[:, t * 128:(t + 1) * 128], qn[:],
                            tbuf[:, t * 128:(t + 1) * 128], OP.add)


# ---------------------------------------------------------------- runner

_CACHE = {}


def kernel(x, edge_index, enc_w, enc_b, ln_g, ln_b, w1, b1, bn_g, bn_b, w2, b2,
           clf_w, clf_b):
    _install_profile_shim()
    x = np.asarray(x, np.float32)
    edge_index = np.asarray(edge_index)
    key = "k"
    if key not in _CACHE:
        prep = preprocess(edge_index)
        has_ln_aff = not (np.allclose(np.asarray(ln_g), 1.0)
                          and np.allclose(np.asarray(ln_b), 0.0))
        nc = build(prep, has_ln_aff)
        _CACHE[key] = (prep, has_ln_aff, nc)
    prep, has_ln_aff, nc = _CACHE[key]

    def col(v):
        return np.asarray(v, np.float32).reshape(-1, 1)

    w1 = np.asarray(w1, np.float32)
    w2 = np.asarray(w2, np.float32)
    b1 = np.asarray(b1, np.float32)
    # weight staging (same for all cores)
    w1s = np.concatenate([w1[i] for i in range(L)], axis=1).astype(BF16)  # [128, L*256]
    w2s = np.zeros((128, L * 256), np.float32)
    for i in range(L):
        w2s[:, i * 256:i * 256 + 128] = np.asarray(w2)[i][0:128, :]
        w2s[:, i * 256 + 128:(i + 1) * 256] = np.asarray(w2)[i][128:256, :]
    b1c = np.zeros((128, L * 2), np.float32)
    b1pv = np.zeros((128, L * 2), np.float32)
    bngv = np.zeros((128, L * 2), np.float32)
    bnbv = np.zeros((128, L * 2), np.float32)
    for i in range(L):
        for hf in range(2):
            b1c[:, i * 2 + hf] = b1[i][hf * 128:(hf + 1) * 128]
            b1pv[:, i * 2 + hf] = 22.0 * b1[i][hf * 128:(hf + 1) * 128]
            bngv[:, i * 2 + hf] = np.asarray(bn_g)[i][hf * 128:(hf + 1) * 128]
            bnbv[:, i * 2 + hf] = np.asarray(bn_b)[i][hf * 128:(hf + 1) * 128]
    b2cv = np.stack([np.asarray(b2)[i] for i in range(L)], axis=1).astype(np.float32)
    lngv = np.zeros((128, L * 128), np.float32)
    lnbv = np.zeros((128, L * 128), np.float32)
    for i in range(L):
        lngv[:, i * 128:(i + 1) * 128] = np.tile(np.asarray(ln_g)[i], (128, 1))
        lnbv[:, i * 128:(i + 1) * 128] = np.tile(np.asarray(ln_b)[i], (128, 1))
    iota = np.tile(np.arange(128, dtype=np.float32), (128, 1))
    ident = np.eye(128, dtype=np.float32)

    common = {
        "zrows": np.zeros((22, 128), np.float32),
        "iota": iota,
        "idbf": ident.astype(BF16),
        "idf": ident,
        "encw": np.asarray(enc_w, np.float32).astype(BF16),
        "encb": col(enc_b),
        "w1": w1s,
        "b1c": b1c, "b1p": b1pv, "bng": bngv, "bnb": bnbv,
        "w2": w2s.astype(BF16),
        "b2c": b2cv,
        "lng": lngv, "lnb": lnbv,
        "clfw": np.asarray(clf_w, np.float32),
        "clfb": col(clf_b),
    }
    in_maps = []
    for o in range(NC):
        xpad = np.zeros((PADN, 128), np.float32)
        xpad[0:PN] = x[o * PN:(o + 1) * PN]
        xs = xpad.T.copy()
        in_maps.append({
            "xT": xs.astype(BF16),
            "idx": prep.idx_wrapped[o],
            "dstc": prep.dstcol[o],
            **common,
        })
    trace = os.environ.get("KERNEL_TRACE", "0") == "1"
    res = run_bass_kernel_spmd(nc, in_maps, list(range(NC)), trace=trace)
    if trace:
        kernel.last_exec_time_ns = res.exec_time_ns
        kernel.last_results = res
    out = np.zeros((N, 64), np.float32)
    for o in range(NC):
        out[o * PN:(o + 1) * PN] = res.results[o]["out"][:, 0:PN].T
    return out


kernel.last_exec_time_ns = None



# revision 16
# speedup vs baseline: 1.6511x; 1.6511x over previous
"""BayesianDeeperGCN on 8 TRN2 NeuronCores (Bass/Tile).

Sharding: nodes across 8 cores (6250 each, padded to 6272 = 49*128);
edges partitioned by destination owner; per layer an AllGather of the
per-node t = relu(LN(h)) table (bf16, 128 wide), then a dma_gather +
per-batch P/Q build + indicator-matmul segmented reduction per dst tile.

Key identity: GENConv softmax aggregation
  agg[d] = sum_e exp(m_e)*m_e / sum_e exp(m_e),  m_e = t[src_e] + eps
(t = relu(LN(h)) >= 0 so no max-subtraction needed) which makes the
whole edge computation two gather-scatter sums over the node table t.

Perf structure: the Q7 SWDGE descriptor generation (~8.6ns/edge) is the
bottleneck, so gather batches are spread round-robin across 4 SWDGE
queues -- queue q's descriptor generation runs on Q7 core pair q, so
the four queues generate concurrently (~1.7x end-to-end). The 128-wide
t table (vs [P|Q] 256-wide) halves the AllGather and gather traffic;
P/Q are built per gathered batch (scalar exp + vector mult).
"""
import os
import sys
import types
import contextlib
import ctypes
import numpy as np
import ml_dtypes

import concourse.bass as bass
import concourse.bacc as bacc
import concourse.tile as tile
import concourse.mybir as mybir
from concourse.bass_utils import run_bass_kernel_spmd

BF16 = ml_dtypes.bfloat16

N = 50000
NC = 8
PN = N // NC            # 6250 nodes per core
NT = 49                 # node tiles of 128 per core
PADN = NT * 128         # 6272
TBLR = NC * PADN        # 50176 padded table rows
LO = 32768              # int16 index split
H = 128
H2 = 256
L = 3
WIN = 4                 # dst tiles per PSUM window
CAP = 16                # max chunks per dma_gather batch
NQB = 4                 # SWDGE queues
PAHEAD = 3              # prepped-but-untriggered batches per queue
GBUFS = 9               # gather buffer slots (also caps total prep-ahead)
EPS_GEN = 1e-7
EPS_NORM = 1e-5
NCHUNKS_512 = [(i * 512, 512) for i in range(12)] + [(6144, 128)]


def _install_profile_shim():
    if "antenv.axon_hooks" in sys.modules:
        return
    so = "/opt/axon/libaxon_pjrt.so"
    hook = None
    if os.path.exists(so):
        lib = ctypes.CDLL(so)
        if hasattr(lib, "axon_start_nrt_profile"):
            lib.axon_start_nrt_profile.argtypes = [ctypes.POINTER(ctypes.c_int64), ctypes.c_size_t]
            lib.axon_start_nrt_profile.restype = ctypes.c_int64
            lib.axon_stop_nrt_profile.argtypes = [ctypes.c_char_p]
            lib.axon_stop_nrt_profile.restype = ctypes.c_int64

            @contextlib.contextmanager
            def _hook(output_dir, device_ids):
                import jax
                jax.devices()
                if device_ids:
                    ids = (ctypes.c_int64 * len(device_ids))(*device_ids)
                    rc = lib.axon_start_nrt_profile(ids, len(device_ids))
                else:
                    rc = lib.axon_start_nrt_profile(None, 0)
                if rc != 0:
                    raise RuntimeError(f"axon_start_nrt_profile rc={rc}")
                try:
                    yield
                finally:
                    n = lib.axon_stop_nrt_profile(str(output_dir).encode())
                    print(f"profile: {n} file(s) -> {output_dir}", file=sys.stderr)

            hook = _hook
    mod = types.ModuleType("antenv.axon_hooks")
    mod.get_axon_ntff_profile_hook = lambda: hook
    mod.set_axon_ntff_profile_hook = lambda h: None
    sys.modules["antenv.axon_hooks"] = mod
    try:
        import antenv
        antenv.axon_hooks = mod
    except ImportError:
        pass


# ---------------------------------------------------------------- host prep

class Prep:
    pass


def preprocess(edge_index):
    src = edge_index[0].astype(np.int64)
    dst = edge_index[1].astype(np.int64)
    owner = dst // PN
    prow = (src // PN) * PADN + (src % PN)

    counts = np.zeros((NC, NT, 2), np.int64)
    per_core = []
    for o in range(NC):
        m = owner == o
        s = prow[m]
        dl = dst[m] - o * PN
        t = dl // 128
        d128 = dl % 128
        g = (s >= LO).astype(np.int64)
        per_core.append((s, t, d128, g))
        for tt in range(NT):
            tm = t == tt
            counts[o, tt, 0] = int(np.sum(tm & (g == 0)))
            counts[o, tt, 1] = int(np.sum(tm & (g == 1)))

    C = np.ceil(counts.max(axis=0) / 128.0).astype(np.int64)  # [NT,2]

    # global chunk order: window-major, grp, tile, chunk
    chunk_meta = []   # (tile, grp)
    slot_base = {}    # (tile,grp) -> first global chunk idx
    for w0 in range(0, NT, WIN):
        tiles = list(range(w0, min(w0 + WIN, NT)))
        for g in (0, 1):
            for t in tiles:
                slot_base[(t, g)] = len(chunk_meta)
                for _ in range(C[t, g]):
                    chunk_meta.append((t, g))
    NCHT = len(chunk_meta)
    NIDX = NCHT * 128

    # first/last chunk per tile (start/stop flags)
    first_chunk = {}
    last_chunk = {}
    for q, (t, g) in enumerate(chunk_meta):
        if t not in first_chunk:
            first_chunk[t] = q
        last_chunk[t] = q

    # batches: consecutive chunks, same grp, same window, <= CAP
    batches = []  # (grp, q0, nq, window_idx)
    q = 0
    while q < NCHT:
        t0, g0 = chunk_meta[q]
        w = t0 // WIN
        q1 = q
        while (q1 < NCHT and chunk_meta[q1][1] == g0
               and chunk_meta[q1][0] // WIN == w and q1 - q < CAP):
            q1 += 1
        batches.append((g0, q, q1 - q, w))
        q = q1

    idx_all = np.zeros((NC, NIDX), np.int16)
    dstc = np.full((NC, NCHT, 128), 200.0, np.float32)
    for o in range(NC):
        s, t, d128, g = per_core[o]
        for tt in range(NT):
            for gg in (0, 1):
                sel = (t == tt) & (g == gg)
                k = int(sel.sum())
                if k == 0:
                    continue
                base = slot_base[(tt, gg)] * 128
                vals = s[sel] - (LO if gg else 0)
                idx_all[o, base:base + k] = vals.astype(np.int16)
                blk = dstc[o].reshape(-1)
                blk[base:base + k] = d128[sel].astype(np.float32)

    p = Prep()
    p.C = C
    p.chunk_meta = chunk_meta
    p.first_chunk = first_chunk
    p.last_chunk = last_chunk
    p.batches = batches
    p.NCHT = NCHT
    p.NIDX = NIDX
    # wrapped idx layout [128, NIDX/16]
    wrapped = np.zeros((NC, 128, NIDX // 16), np.int16)
    for o in range(NC):
        w16 = idx_all[o].reshape(NIDX // 16, 16).T
        wrapped[o] = np.tile(w16, (8, 1))
    p.idx_wrapped = wrapped
    p.dstcol = np.transpose(dstc, (0, 2, 1)).copy()  # [NC,128,NCHT]
    return p


# ---------------------------------------------------------------- kernel build

class EdgeState:
    """Per-layer prepare/trigger bookkeeping for the gather batches."""

    def __init__(self, nc, prep, gpool, idx, lo_view, hi_view, qsems, bf):
        self.nc = nc
        self.prep = prep
        self.gpool = gpool
        self.idx = idx
        self.lo_view = lo_view
        self.hi_view = hi_view
        self.qsems = qsems
        self.bf = bf
        self.next = 0
        self.pending = [[] for _ in range(NQB)]
        self.gbs = {}
        self.fired = set()

    def _emit_prep(self):
        b = self.next
        g0, c0, nq, w = self.prep.batches[b]
        q = b % NQB if os.environ.get("KERNEL_1Q", "0") != "1" else 0
        gb = self.gpool.tile([128, nq, 128], self.bf, tag="gath")
        src_view = self.hi_view if g0 else self.lo_view
        if os.environ.get("KERNEL_PREP", "0") == "1":
            self.nc.gpsimd.dma_gather(
                gb[:], src_view, self.idx[:, c0 * 8:(c0 + nq) * 8],
                nq * 128, nq * 128, 128,
                single_packet=False, queue_num=q,
                prepare_only=True, sem=self.qsems[q])
            self.pending[q].append(b)
        else:
            self.nc.gpsimd.dma_gather(
                gb[:], src_view, self.idx[:, c0 * 8:(c0 + nq) * 8],
                nq * 128, nq * 128, 128,
                single_packet=False, queue_num=q)
            self.fired.add(b)
        self.gbs[b] = gb
        self.next += 1

    def pump(self):
        if os.environ.get("KERNEL_PREP", "0") != "1":
            return
        nb = len(self.prep.batches)
        while (self.next < nb and len(self.gbs) < GBUFS
               and len(self.pending[self.next % NQB]) < PAHEAD):
            self._emit_prep()

    def ensure_fired(self, b):
        if os.environ.get("KERNEL_PREP", "0") != "1":
            while self.next <= b:
                self._emit_prep()
            return
        if b in self.fired:
            return
        q = b % NQB if os.environ.get("KERNEL_1Q", "0") != "1" else 0
        assert b in self.pending[q]
        self.nc.gpsimd.trigger_dma(count=None, queue_num=q)
        self.fired.update(self.pending[q])
        self.pending[q].clear()
        self.pump()


def build(prep, has_ln_aff):
    CUT = int(os.environ.get("KERNEL_CUT", "0"))
    f32 = mybir.dt.float32
    bf = mybir.dt.bfloat16
    nc = bacc.Bacc(None, num_swdge_queues=NQB, dynamic_dma_scratch_size=32768)
    AF = mybir.ActivationFunctionType
    OP = mybir.AluOpType

    def din(name, shape, dt=f32):
        return nc.dram_tensor(name, shape, dt, kind="ExternalInput")

    xT_in = din("xT", [128, PADN], bf)
    idx_in = din("idx", [128, prep.NIDX // 16], mybir.dt.int16)
    dstc_in = din("dstc", [128, prep.NCHT])
    iota_in = din("iota", [128, 128])
    idbf_in = din("idbf", [128, 128], bf)
    idf_in = din("idf", [128, 128])
    encw_in = din("encw", [128, 128], bf)
    encb_in = din("encb", [128, 1])
    w1_in = din("w1", [128, L * 256], bf)
    b1_in = din("b1c", [128, L * 2])
    b1p_in = din("b1p", [128, L * 2])
    bng_in = din("bng", [128, L * 2])
    bnb_in = din("bnb", [128, L * 2])
    w2_in = din("w2", [128, L * 256], bf)
    b2_in = din("b2c", [128, L])
    lng_in = din("lng", [128, L * 128])
    lnb_in = din("lnb", [128, L * 128])
    clfw_in = din("clfw", [128, 64])
    clfb_in = din("clfb", [64, 1])
    out_ext = nc.dram_tensor("out", [64, PADN], f32, kind="ExternalOutput")

    bounce = nc.dram_tensor("bounce", [PADN, 128], bf)
    table = nc.dram_tensor("table", [TBLR, 128], bf, addr_space="Shared")
    bnloc = nc.dram_tensor("bnloc", [128, 4], f32)
    zrows = nc.dram_tensor("zrows", [22, 128], f32, kind="ExternalInput")
    bnred = nc.dram_tensor("bnred", [128, 4], f32, addr_space="Shared")

    RG = [list(range(NC))]

    with tile.TileContext(nc) as tc:
        with (
            tc.tile_pool(name="const", bufs=1) as cpool,
            tc.tile_pool(name="state", bufs=1) as spool,
            tc.tile_pool(name="gathp", bufs=GBUFS) as gpool,
            tc.tile_pool(name="pqp", bufs=2) as qpool,
            tc.tile_pool(name="indp", bufs=2) as ipool,
            tc.tile_pool(name="work", bufs=3) as wpool,
            tc.tile_pool(name="tiny", bufs=2) as ypool,
            tc.tile_pool(name="agg", bufs=WIN, space="PSUM") as agg_pool,
            tc.tile_pool(name="mm", bufs=2, space="PSUM") as mm_pool,
            tc.tile_pool(name="tr", bufs=2, space="PSUM") as tr_pool,
        ):
            def load(src_ap, shape, dt=f32, pool=cpool, name=None):
                t = pool.tile(shape, dt, name=name, tag=name)
                nc.sync.dma_start(t[:], src_ap)
                return t

            idx = load(idx_in[:], [128, prep.NIDX // 16], mybir.dt.int16, name="idx")
            dstc = load(dstc_in[:], [128, prep.NCHT], name="dstc")
            iota = load(iota_in[:], [128, 128], name="iota")
            idbf = load(idbf_in[:], [128, 128], bf, name="idbf")
            idf = load(idf_in[:], [128, 128], name="idf")
            encw = load(encw_in[:], [128, 128], bf, name="encw")
            encb = load(encb_in[:], [128, 1], name="encb")
            w1 = load(w1_in[:], [128, L * 256], bf, name="w1")
            b1c = load(b1_in[:], [128, L * 2], name="b1c")
            b1p = load(b1p_in[:], [128, L * 2], name="b1p")
            bng = load(bng_in[:], [128, L * 2], name="bng")
            bnb = load(bnb_in[:], [128, L * 2], name="bnb")
            w2 = load(w2_in[:], [128, L * 256], bf, name="w2")
            b2c = load(b2_in[:], [128, L], name="b2c")
            lng = load(lng_in[:], [128, L * 128], name="lng") if has_ln_aff else None
            lnb = load(lnb_in[:], [128, L * 128], name="lnb") if has_ln_aff else None
            clfw = load(clfw_in[:], [128, 64], name="clfw")
            clfb = load(clfb_in[:], [64, 1], name="clfb")

            xT = spool.tile([128, 2 * PADN], bf, tag="h2T", name="xT")
            nc.sync.dma_start(xT[:, 0:PADN], xT_in[:])
            epsn_c = cpool.tile([128, 1], f32)
            nc.vector.memset(epsn_c[:], EPS_NORM)
            epsg_c = cpool.tile([128, 1], f32)
            nc.vector.memset(epsg_c[:], EPS_GEN)
            h = spool.tile([128, PADN], f32)
            tbuf = spool.tile([128, PADN], f32, tag="tbuf")
            hc = spool.tile([128, PADN], bf)
            hcT = spool.tile([128, PADN], bf)
            h2T = spool.tile([128, 2 * PADN], bf)
            stats6 = spool.tile([128, NT * 6], f32)

            lo_view = table[0:LO, :]
            hi_view = table[LO:TBLR, :]
            qsems = [nc.alloc_semaphore(f"gdma{q}") for q in range(NQB)]
            NL = 0 if CUT == 1 else (1 if CUT in (2, 3) else L)
            edges = [EdgeState(nc, prep, gpool, idx, lo_view, hi_view, qsems, bf)
                     for _ in range(NL)]

            # prep-ahead for layer 0 runs during input load + encoder
            if NL > 0 and CUT != 2:
                edges[0].pump()

            # ---------------- encoder: h0T = encw.T @ xT ; transpose to h
            sc_enc = nc.enter_named_scope("enc", False)
            for (c0, cn) in NCHUNKS_512:
                ps = mm_pool.tile([128, 512], f32, tag="mm")
                nc.tensor.matmul(ps[:, 0:cn], encw[:], xT[:, 0:PADN][:, c0:c0 + cn],
                                 start=True, stop=True)
                nc.vector.tensor_scalar(tbuf[:, c0:c0 + cn], ps[:, 0:cn],
                                        encb[:, 0:1], None, OP.add)
            for t in range(NT):
                pt = tr_pool.tile([128, 128], f32, tag="tr")
                nc.tensor.transpose(pt[:], tbuf[:, t * 128:(t + 1) * 128], idf[:])
                nc.vector.tensor_copy(h[:, t * 128:(t + 1) * 128], pt[:])
            nc.sync.dma_start(h[106:128, 48 * 128:PADN], zrows[:])
            nc.leave_named_scope("enc", sc_enc[0], False)

            # ---------------- layers
            for li in range(NL):
                sc_a = nc.enter_named_scope(f"L{li}_ln_pq", False)
                # ---- A1: LN stats per tile
                for t in range(NT):
                    nc.vector.bn_stats(stats6[:, t * 6:(t + 1) * 6],
                                       h[:, t * 128:(t + 1) * 128])
                # ---- A2: batched mu/rstd
                def sview(k):
                    return (stats6[:].rearrange("p (t s) -> p t s", s=6)
                            [:, :, k:k + 1].rearrange("p t s -> p (t s)"))
                me, mo, m2e, m2o = sview(1), sview(4), sview(2), sview(5)
                musum = ypool.tile([128, NT], f32, tag="y1")
                nc.vector.tensor_tensor(musum[:], me, mo, OP.add)
                mu_all = ypool.tile([128, NT], f32, tag="y2")
                nc.vector.tensor_scalar(mu_all[:], musum[:], 0.5, None, OP.mult)
                mesq = ypool.tile([128, NT], f32, tag="y3")
                nc.vector.tensor_tensor(mesq[:], me, me, OP.mult)
                mosq = ypool.tile([128, NT], f32, tag="y4")
                nc.vector.tensor_tensor(mosq[:], mo, mo, OP.mult)
                sqs = ypool.tile([128, NT], f32, tag="y5")
                nc.vector.tensor_tensor(sqs[:], mesq[:], mosq[:], OP.add)
                m2s = ypool.tile([128, NT], f32, tag="y3")
                nc.vector.tensor_tensor(m2s[:], m2e, m2o, OP.add)
                # sumsq = m2s + 64*sqs ; E2 = sumsq/128 ; var = E2 - mu^2
                sq64 = ypool.tile([128, NT], f32, tag="y4")
                nc.vector.tensor_scalar(sq64[:], sqs[:], 64.0, None, OP.mult)
                ssq = ypool.tile([128, NT], f32, tag="y5")
                nc.vector.tensor_tensor(ssq[:], m2s[:], sq64[:], OP.add)
                musq = ypool.tile([128, NT], f32, tag="y3")
                nc.vector.tensor_tensor(musq[:], mu_all[:], mu_all[:], OP.mult)
                var = ypool.tile([128, NT], f32, tag="y4")
                nc.vector.tensor_scalar(var[:], ssq[:], 1.0 / 128, None, OP.mult)
                nc.vector.tensor_tensor(var[:], var[:], musq[:], OP.subtract)
                lnv = ypool.tile([128, NT], f32, tag="y5")
                nc.scalar.activation(lnv[:], var[:], AF.Ln, bias=epsn_c[:, 0:1])
                lnv2 = ypool.tile([128, NT], f32, tag="y3")
                nc.vector.tensor_scalar(lnv2[:], lnv[:], -0.5, None, OP.mult)
                rstd = ypool.tile([128, NT], f32, tag="y6")
                nc.scalar.activation(rstd[:], lnv2[:], AF.Exp)

                # ---- A3: wide t = relu(norm)
                muv = mu_all[:].rearrange("p (t o) -> p t o", o=1).broadcast_to((128, NT, 128))
                rsv = rstd[:].rearrange("p (t o) -> p t o", o=1).broadcast_to((128, NT, 128))
                hv = h[:].rearrange("p (t j) -> p t j", j=128)
                tv = tbuf[:].rearrange("p (t j) -> p t j", j=128)
                nc.vector.tensor_tensor(tv, hv, muv, OP.subtract)
                nc.vector.tensor_tensor(tv, tv, rsv, OP.mult)
                if has_ln_aff:
                    lgv = (lng[:, li * 128:(li + 1) * 128]
                           .rearrange("p (o j) -> p o j", o=1).broadcast_to((128, NT, 128)))
                    lbv = (lnb[:, li * 128:(li + 1) * 128]
                           .rearrange("p (o j) -> p o j", o=1).broadcast_to((128, NT, 128)))
                    nc.vector.tensor_tensor(tv, tv, lgv, OP.mult)
                    nc.vector.tensor_tensor(tv, tv, lbv, OP.add)
                nc.vector.tensor_scalar(tbuf[:], tbuf[:], 0.0, None, OP.max)
                # ---- A4: per-tile bf16 cast + bounce write (sync/HWDGE)
                for t in range(NT):
                    st = wpool.tile([128, 128], bf, tag="tcast")
                    nc.vector.tensor_copy(st[:], tbuf[:, t * 128:(t + 1) * 128])
                    nc.sync.dma_start(bounce[t * 128:(t + 1) * 128, :], st[:])
                nc.leave_named_scope(f"L{li}_ln_pq", sc_a[0], False)

                # ---- B: prep-ahead for this layer's gathers, then AllGather
                sc_b = nc.enter_named_scope(f"L{li}_ag", False)
                edges[li].pump()
                nc.gpsimd.collective_compute(
                    "AllGather", OP.bypass, replica_groups=RG,
                    ins=[bounce[:].opt()], outs=[table[:].opt()])
                nc.leave_named_scope(f"L{li}_ag", sc_b[0], False)

                # ---- C: edge phase
                if CUT == 2:
                    break
                sc_c = nc.enter_named_scope(f"L{li}_edge", False)
                st_ = edges[li]
                aggps = {}
                cur_w = -1
                for b, (g0, c0, nq, w) in enumerate(prep.batches):
                    if w != cur_w:
                        if cur_w >= 0:
                            for t in range(cur_w * WIN, min((cur_w + 1) * WIN, NT)):
                                _drain(nc, tc, wpool, aggps[t], tbuf, hc, t)
                        aggps = {}
                        cur_w = w
                        for t in range(w * WIN, min((w + 1) * WIN, NT)):
                            aggps[t] = agg_pool.tile([128, 256], f32, tag="agg",
                                                     name=f"agg{t % WIN}")
                    st_.ensure_fired(b)
                    gb = st_.gbs.pop(b)
                    PQMM = os.environ.get("KERNEL_PQMM", "pq")
                    if PQMM == "pq":
                        pqc = qpool.tile([128, nq, 256], bf, tag="pc")
                        gv = gb[:]
                        nc.scalar.activation(pqc[:, :, 0:128], gv, AF.Exp,
                                             bias=epsg_c[:, 0:1])
                        nc.vector.tensor_tensor(pqc[:, :, 128:256],
                                                pqc[:, :, 0:128], gv, OP.mult)
                    else:
                        pc = qpool.tile([128, nq * 128], bf, tag="pc")
                        nc.scalar.activation(pc[:], gb[:].rearrange("p q j -> p (q j)"),
                                             AF.Exp, bias=epsg_c[:, 0:1])
                        qc = qpool.tile([128, nq * 128], bf, tag="qc")
                        nc.vector.tensor_tensor(qc[:], pc[:],
                                                gb[:].rearrange("p q j -> p (q j)"),
                                                OP.mult)
                    ib = ipool.tile([128, nq * 128], bf, tag="ind")
                    dv = (dstc[:, c0:c0 + nq].rearrange("p (q o) -> p q o", o=1)
                          .broadcast_to((128, nq, 128)))
                    iv = (iota[:].rearrange("p (o j) -> p o j", o=1)
                          .broadcast_to((128, nq, 128)))
                    ibv = ib[:].rearrange("p (q j) -> p q j", j=128)
                    nc.vector.tensor_tensor(ibv, dv, iv, OP.is_equal)
                    for ci in range(nq):
                        qch = c0 + ci
                        t = prep.chunk_meta[qch][0]
                        if PQMM == "pq":
                            nc.tensor.matmul(aggps[t][:], ib[:, ci * 128:(ci + 1) * 128],
                                             pqc[:, ci, :],
                                             start=(qch == prep.first_chunk[t]),
                                             stop=(qch == prep.last_chunk[t]))
                        else:
                            nc.tensor.matmul(aggps[t][:, 0:128],
                                             ib[:, ci * 128:(ci + 1) * 128],
                                             pc[:, ci * 128:(ci + 1) * 128],
                                             start=(qch == prep.first_chunk[t]),
                                             stop=(qch == prep.last_chunk[t]))
                            nc.tensor.matmul(aggps[t][:, 128:256],
                                             ib[:, ci * 128:(ci + 1) * 128],
                                             qc[:, ci * 128:(ci + 1) * 128],
                                             start=(qch == prep.first_chunk[t]),
                                             stop=(qch == prep.last_chunk[t]))
                for t in range(cur_w * WIN, min((cur_w + 1) * WIN, NT)):
                    _drain(nc, tc, wpool, aggps[t], tbuf, hc, t)
                nc.leave_named_scope(f"L{li}_edge", sc_c[0], False)

                # ---- D: dense tail
                if CUT == 3:
                    break
                sc_d = nc.enter_named_scope(f"L{li}_tail", False)
                for t in range(NT):
                    pt = tr_pool.tile([128, 128], bf, tag="tr", name="ptb")
                    nc.tensor.transpose(pt[:], hc[:, t * 128:(t + 1) * 128], idbf[:])
                    nc.vector.tensor_copy(hcT[:, t * 128:(t + 1) * 128], pt[:])
                accC = ypool.tile([128, 26], f32, tag="acc")
                for hf in range(2):
                    lhs = w1[:, li * 256 + hf * 128: li * 256 + (hf + 1) * 128]
                    for i, (c0, cn) in enumerate(NCHUNKS_512):
                        ps = mm_pool.tile([128, 512], f32, tag="mm")
                        nc.tensor.matmul(ps[:, 0:cn], lhs, hcT[:, c0:c0 + cn],
                                         start=True, stop=True)
                        nc.vector.tensor_scalar(
                            h2T[:, hf * PADN + c0: hf * PADN + c0 + cn],
                            ps[:, 0:cn], b1c[:, li * 2 + hf: li * 2 + hf + 1],
                            0.0, OP.add, OP.add,
                            accum_out=accC[:, hf * 13 + i: hf * 13 + i + 1])
                    nc.vector.memset(h2T[:, hf * PADN + 6250: (hf + 1) * PADN], 0.0)
                # BN stats
                st4 = ypool.tile([128, 4], f32, tag="st4")
                for hf in range(2):
                    sumh = ypool.tile([128, 1], f32, tag="y1")
                    nc.vector.tensor_reduce(sumh[:], accC[:, hf * 13:(hf + 1) * 13],
                                            mybir.AxisListType.X, OP.add)
                    nc.vector.tensor_tensor(st4[:, hf:hf + 1], sumh[:],
                                            b1p[:, li * 2 + hf: li * 2 + hf + 1],
                                            OP.subtract)
                    nc.scalar.activation(hcT[:], h2T[:, hf * PADN:(hf + 1) * PADN],
                                         AF.Square,
                                         accum_out=st4[:, 2 + hf:3 + hf])
                nc.sync.dma_start(bnloc[:], st4[:])
                nc.gpsimd.collective_compute(
                    "AllReduce", OP.add, replica_groups=RG,
                    ins=[bnloc[:].opt()], outs=[bnred[:].opt()])
                gst = ypool.tile([128, 4], f32, tag="gst")
                nc.sync.dma_start(gst[:], bnred[:])
                for hf in range(2):
                    mu = ypool.tile([128, 1], f32, tag="y1")
                    nc.vector.tensor_scalar(mu[:], gst[:, hf:hf + 1], 1.0 / N, None, OP.mult)
                    musq = ypool.tile([128, 1], f32, tag="y2")
                    nc.vector.tensor_tensor(musq[:], mu[:], mu[:], OP.mult)
                    var = ypool.tile([128, 1], f32, tag="y3")
                    nc.vector.tensor_scalar(var[:], gst[:, 2 + hf:3 + hf], 1.0 / N,
                                            musq[:], OP.mult, OP.subtract)
                    lnv = ypool.tile([128, 1], f32, tag="y4")
                    nc.scalar.activation(lnv[:], var[:], AF.Ln, bias=epsn_c[:, 0:1])
                    lnv2 = ypool.tile([128, 1], f32, tag="y5")
                    nc.vector.tensor_scalar(lnv2[:], lnv[:], -0.5, None, OP.mult)
                    rs = ypool.tile([128, 1], f32, tag="y4")
                    nc.scalar.activation(rs[:], lnv2[:], AF.Exp)
                    a = ypool.tile([128, 1], f32, tag="y5")
                    nc.vector.tensor_tensor(a[:], bng[:, li * 2 + hf: li * 2 + hf + 1],
                                            rs[:], OP.mult)
                    amu = ypool.tile([128, 1], f32, tag="y4")
                    nc.vector.tensor_tensor(amu[:], a[:], mu[:], OP.mult)
                    bp = ypool.tile([128, 1], f32, tag="y6")
                    nc.vector.tensor_tensor(bp[:], bnb[:, li * 2 + hf: li * 2 + hf + 1],
                                            amu[:], OP.subtract)
                    nc.scalar.activation(h2T[:, hf * PADN:(hf + 1) * PADN],
                                         h2T[:, hf * PADN:(hf + 1) * PADN],
                                         AF.Relu, bias=bp[:, 0:1], scale=a[:, 0:1])
                # w2
                for i, (c0, cn) in enumerate(NCHUNKS_512):
                    ps = mm_pool.tile([128, 512], f32, tag="mm")
                    for cc in range(2):
                        lhs = w2[:, li * 256 + cc * 128: li * 256 + (cc + 1) * 128]
                        nc.tensor.matmul(ps[:, 0:cn], lhs,
                                         h2T[:, cc * PADN + c0: cc * PADN + c0 + cn],
                                         start=(cc == 0), stop=(cc == 1))
                    nc.vector.tensor_scalar(tbuf[:, c0:c0 + cn], ps[:, 0:cn],
                                            b2c[:, li:li + 1], None, OP.add)
                for t in range(NT):
                    pt = tr_pool.tile([128, 128], f32, tag="tr")
                    nc.tensor.transpose(pt[:], tbuf[:, t * 128:(t + 1) * 128], idf[:])
                    nc.vector.tensor_tensor(h[:, t * 128:(t + 1) * 128],
                                            h[:, t * 128:(t + 1) * 128], pt[:], OP.add)
                nc.sync.dma_start(h[106:128, 48 * 128:PADN], zrows[:])
                nc.leave_named_scope(f"L{li}_tail", sc_d[0], False)

            # ---------------- classifier
            for t in range(NT):
                pt = tr_pool.tile([128, 128], f32, tag="tr")
                nc.tensor.transpose(pt[:], h[:, t * 128:(t + 1) * 128], idf[:])
                nc.vector.tensor_copy(tbuf[:, t * 128:(t + 1) * 128], pt[:])
            ob = spool.tile([64, PADN], f32, tag="h2T", name="ob")
            for (c0, cn) in NCHUNKS_512:
                ps = mm_pool.tile([64, 512], f32, tag="mm", name="psclf")
                nc.tensor.matmul(ps[:, 0:cn], clfw[:], tbuf[:, c0:c0 + cn],
                                 start=True, stop=True)
                nc.vector.tensor_scalar(ob[:, c0:c0 + cn], ps[:, 0:cn],
                                        clfb[:, 0:1], None, OP.add)
            nc.sync.dma_start(out_ext[:], ob[:])
    nc.compile()
    return nc


def _drain(nc, tc, wpool, ps, tbuf, hc, t):
    OP = mybir.AluOpType
    f32 = mybir.dt.float32
    den = wpool.tile([128, 128], f32, tag="den")
    nc.vector.tensor_scalar(den[:], ps[:, 0:128], 1e-20, None, OP.add)
    r = wpool.tile([128, 128], f32, tag="rcp")
    nc.vector.reciprocal_approx_fast(r[:], den[:])
    qn = wpool.tile([128, 128], f32, tag="qn")
    nc.vector.tensor_tensor(qn[:], ps[:, 128:256], r[:], OP.mult)
    nc.vector.tensor_tensor(hc[:, t * 128:(t + 1) * 128], qn[:],
                            tbuf[:, t * 128:(t + 1) * 128], OP.add)


# ---------------------------------------------------------------- runner

_CACHE = {}


def kernel(x, edge_index, enc_w, enc_b, ln_g, ln_b, w1, b1, bn_g, bn_b, w2, b2,
           clf_w, clf_b):
    _install_profile_shim()
    x = np.asarray(x, np.float32)
    edge_index = np.asarray(edge_index)
    key = "k"
    if key not in _CACHE:
        prep = preprocess(edge_index)
        has_ln_aff = not (np.allclose(np.asarray(ln_g), 1.0)
                          and np.allclose(np.asarray(ln_b), 0.0))
        nc = build(prep, has_ln_aff)
        _CACHE[key] = (prep, has_ln_aff, nc)
    prep, has_ln_aff, nc = _CACHE[key]

    def col(v):
        return np.asarray(v, np.float32).reshape(-1, 1)

    w1 = np.asarray(w1, np.float32)
    w2 = np.asarray(w2, np.float32)
    b1 = np.asarray(b1, np.float32)
    # weight staging (same for all cores)
    w1s = np.concatenate([w1[i] for i in range(L)], axis=1).astype(BF16)  # [128, L*256]
    w2s = np.zeros((128, L * 256), np.float32)
    for i in range(L):
        w2s[:, i * 256:i * 256 + 128] = np.asarray(w2)[i][0:128, :]
        w2s[:, i * 256 + 128:(i + 1) * 256] = np.asarray(w2)[i][128:256, :]
    b1c = np.zeros((128, L * 2), np.float32)
    b1pv = np.zeros((128, L * 2), np.float32)
    bngv = np.zeros((128, L * 2), np.float32)
    bnbv = np.zeros((128, L * 2), np.float32)
    for i in range(L):
        for hf in range(2):
            b1c[:, i * 2 + hf] = b1[i][hf * 128:(hf + 1) * 128]
            b1pv[:, i * 2 + hf] = 22.0 * b1[i][hf * 128:(hf + 1) * 128]
            bngv[:, i * 2 + hf] = np.asarray(bn_g)[i][hf * 128:(hf + 1) * 128]
            bnbv[:, i * 2 + hf] = np.asarray(bn_b)[i][hf * 128:(hf + 1) * 128]
    b2cv = np.stack([np.asarray(b2)[i] for i in range(L)], axis=1).astype(np.float32)
    lngv = np.zeros((128, L * 128), np.float32)
    lnbv = np.zeros((128, L * 128), np.float32)
    for i in range(L):
        lngv[:, i * 128:(i + 1) * 128] = np.tile(np.asarray(ln_g)[i], (128, 1))
        lnbv[:, i * 128:(i + 1) * 128] = np.tile(np.asarray(ln_b)[i], (128, 1))
    iota = np.tile(np.arange(128, dtype=np.float32), (128, 1))
    ident = np.eye(128, dtype=np.float32)

    common = {
        "zrows": np.zeros((22, 128), np.float32),
        "iota": iota,
        "idbf": ident.astype(BF16),
        "idf": ident,
        "encw": np.asarray(enc_w, np.float32).astype(BF16),
        "encb": col(enc_b),
        "w1": w1s,
        "b1c": b1c, "b1p": b1pv, "bng": bngv, "bnb": bnbv,
        "w2": w2s.astype(BF16),
        "b2c": b2cv,
        "lng": lngv, "lnb": lnbv,
        "clfw": np.asarray(clf_w, np.float32),
        "clfb": col(clf_b),
    }
    in_maps = []
    for o in range(NC):
        xpad = np.zeros((PADN, 128), np.float32)
        xpad[0:PN] = x[o * PN:(o + 1) * PN]
        xs = xpad.T.copy()
        in_maps.append({
            "xT": xs.astype(BF16),
            "idx": prep.idx_wrapped[o],
            "dstc": prep.dstcol[o],
            **common,
        })
    trace = os.environ.get("KERNEL_TRACE", "0") == "1"
    res = run_bass_kernel_spmd(nc, in_maps, list(range(NC)), trace=trace)
    if trace:
        kernel.last_exec_time_ns = res.exec_time_ns
        kernel.last_results = res
    out = np.zeros((N, 64), np.float32)
    for o in range(NC):
        out[o * PN:(o + 1) * PN] = res.results[o]["out"][:, 0:PN].T
    return out


kernel.last_exec_time_ns = None
